# revision 9
# baseline (speedup 1.0000x reference)
"""Locally-connected conv (LocalLinear) Trainium2 Bass kernel.

Problem: x (B=64, Cin=64, 32, 32), weight (Cout=64, Cin=64, 32, 32, 3, 3),
bias (Cout=64, 32, 32) -> out (B=64, Cout=64, 32, 32).
out[b,o,y,x] = sum_{c,u,v} xpad[b,c,y+u-1,x+v-1] * W[o,c,y,x,u,v] + bias[o,y,x]

Sharding: spatial rows across 8 cores (core i owns output rows [4i,4i+4)).

v2 design (vs the 62us fp16 baseline):
  - weights ship as fp8 e3m4 (4 mantissa bits): rel err measured 1.4e-2
    vs the 2e-2 gate; weight DMA halves to 4.72MB/core.  x stays fp16.
  - x ships ONCE (1.67MB): xh[c + 64*(k%2), k//2, R, b] packs even
    padded-columns k in partitions 0-63, odd in 64-127.
  - stationary store X2 [128, R=6, 34*64]: padded column k occupies free
    cols [64k,64k+64) on partition half (k%2); the complement half of
    every column block stays ZERO (pre-zeroed once, split across
    gpsimd/ACT/DVE so no engine serializes the start).  The [128,128]
    stationary for (C,R) is then just the contiguous window
    X2[:, R, 64C:64C+128] - block-diagonal by construction, and each
    x value is stored once (DVE copy volume halved vs the xz scheme).
  - weights are host-gathered into wm3 [128, 36864] fp8 in EXACT matmul
    consumption order: for C ascending, R, v: one moving block
    [128, n*64] covering all valid yy=R-u rows at once (n<=3).  One
    matmul per (C,R,v) - 288 instead of 576 - each streaming a plain
    contiguous wm3 slice.  Moving rows: the weight for tap v of the
    even location sits on partition half (v%2), odd location on the
    complement - matching where X2 holds the corresponding x column.
  - psum: one full bank [128, 8, 64] fp32 per location-pair xp (yy in
    slots 0-3); start=True on the first (C=2xp,R=0) matmul zeroes the
    bank (PSUM zero-region = 2KB), stop on the last (C=2xp+2,R=5).
    One batched ACT drain [128,4,64] per xp (16 drains, not 64).
  - output DMAs ride the gpsimd queue (25ns dispatch, engine idle).
  - input DMAs on the sync queue in consumption order, xh interleaved
    with wm3 column slabs.
"""

import numpy as np
import ml_dtypes

import concourse.bacc as bacc
import concourse.mybir as mybir
import concourse.tile as tile
from concourse.bass_utils import run_bass_kernel_spmd

NCORES = 8
B = 64
CIN = 64
COUT = 64
H = 32
ROWS = 4            # output rows per core
NXP = 16            # location-pairs per row (x paired even/odd)
NCOL = 33           # stationary windows C in [0, 33)
NK = 34             # padded x columns k in [0, 34)
NJ = 17             # xh slots per parity half

F16 = mybir.dt.float16
F8 = mybir.dt.float8e3
F32 = mybir.dt.float32
NP_F8 = ml_dtypes.float8_e3m4

X2W = NK * 64       # X2 free columns (2176)


def _yys(R):
    return range(max(0, R - 2), min(3, R) + 1)


def _blocks():
    """Matmul stream: list of (C, R, v, xp, y0, n, col_off) in order."""
    out = []
    off = 0
    for C in range(NCOL):
        for R in range(6):
            for v in ((0, 2) if C % 2 == 0 else (1,)):
                xp = (C - v) // 2
                if not (0 <= xp < NXP):
                    continue
                ys = list(_yys(R))
                n = len(ys)
                out.append((C, R, v, xp, ys[0], n, off))
                off += n * 64
    return out, off


BLOCKS, WCOLS = _blocks()          # WCOLS == 36864

# input DMA interleave: xh is small (1.67MB) and gates the DVE->PE chain,
# so it is front-loaded; weight slabs stream behind in consumption order.


def _wcol_bound(cbound):
    for (C, R, v, xp, y0, n, off) in BLOCKS:
        if C >= cbound:
            return off
    return WCOLS


_W_SLICES = []
_prev = 0
for cb in (2, 6, 10, 14, 19, 24, 33):
    b = _wcol_bound(cb)
    _W_SLICES.append((_prev, b))
    _prev = b

_nc_cache = None


def _build_nc():
    from contextlib import ExitStack

    nc = bacc.Bacc("TRN2", target_bir_lowering=False)

    xh_d = nc.dram_tensor("xh", [128, NJ, 6, B], F16, kind="ExternalInput")
    w_d = nc.dram_tensor("wm", [128, WCOLS], F8, kind="ExternalInput")
    # out: [4 groups of 4 xp, 128 (b | b+64 odd loc), 16 slots, 64 o] fp16
    o_d = nc.dram_tensor("out_p", [4, 128, 16, COUT], F16, kind="ExternalOutput")

    with tile.TileContext(nc) as tc, ExitStack() as ctx:
        xhpool = ctx.enter_context(tc.tile_pool(name="xhp", bufs=1))
        x2pool = ctx.enter_context(tc.tile_pool(name="x2p", bufs=1))
        wpool = ctx.enter_context(tc.tile_pool(name="wp", bufs=1))
        zpool = ctx.enter_context(tc.tile_pool(name="zp", bufs=1))
        opool = ctx.enter_context(tc.tile_pool(name="op", bufs=4))
        pspool = ctx.enter_context(tc.tile_pool(name="ps", bufs=8, space="PSUM"))

        xh_sb = xhpool.tile([128, NJ, 6, B], F16, name="xh")
        w_sb = wpool.tile([128, WCOLS], F8, name="wm3")
        x2 = x2pool.tile([128, 6, X2W], F16, name="x2")
        zt = zpool.tile([128, 1], F16, name="zt")

        # pre-zero X2: DVE does the front (finishes before the first x
        # copies need the queue), ACT broadcast-copies the middle+tail from
        # a zero tile.  gpsimd is kept OFF here: concurrent gpsimd memsets
        # were measured to slow overlapping DVE ops ~8x.
        nc.vector.memset(zt[:], 0.0)
        nc.vector.memset(x2[:, :, 0:512], 0.0)
        nc.scalar.copy(x2[:, :, 512:1344], zt[:].to_broadcast([128, 6, 832]))
        nc.scalar.copy(x2[:, :, 1344:X2W], zt[:].to_broadcast([128, 6, 832]))

        def copy_slice(j0, j1):
            # xh slots [j0,j1) -> X2 data blocks (even half then odd half)
            nj = j1 - j0
            for h in (0, 1):
                p0 = 64 * h
                src = xh_sb[p0:p0 + 64, j0:j1, :, :]
                dst_ap = x2[p0:p0 + 64, :, :]
                # custom view: [64p, nj (stride 128), 6R (stride X2W), 64b]
                import concourse.ap as cap
                base = x2[p0:p0 + 64, 0, 128 * j0 + 64 * h]
                ap = cap.AP(
                    base.tensor, base.offset,
                    [list(base.ap[0]), [128, nj], [X2W, 6], [1, 64]],
                )
                nc.vector.tensor_scalar_add(ap, src, 0.0)

        # input stream on sync queue: xh front-loaded, weights behind
        def wdma(k):
            a, b = _W_SLICES[k]
            nc.sync.dma_start(w_sb[:, a:b], w_d[:, a:b])

        nc.sync.dma_start(xh_sb[:, 0:1], xh_d[:, 0:1])
        copy_slice(0, 1)
        wdma(0)
        nc.sync.dma_start(xh_sb[:, 1:4], xh_d[:, 1:4])
        copy_slice(1, 4)
        wdma(1)
        wdma(2)
        nc.sync.dma_start(xh_sb[:, 4:10], xh_d[:, 4:10])
        copy_slice(4, 10)
        wdma(3)
        nc.sync.dma_start(xh_sb[:, 10:17], xh_d[:, 10:17])
        copy_slice(10, 17)
        wdma(4)
        wdma(5)
        wdma(6)

        ps = {}
        out_sb = [None] * 4
        done_in_group = [0] * 4

        for (C, R, v, xp, y0, n, off) in BLOCKS:
            stat = x2[:, R, 64 * C:64 * C + 128]
            mov = w_sb[:, off:off + n * 64]
            first = (v == 0 and R == 0)
            last = (v == 2 and R == 5)
            if first:
                ps[xp] = pspool.tile([128, 8, COUT], F32, name="pst")
            out_ap = ps[xp][:, y0:y0 + n, :]
            nc.tensor.matmul(out_ap, stat, mov, start=first, stop=last)
            if last:
                g, gs = divmod(xp, 4)
                if done_in_group[g] == 0:
                    out_sb[g] = opool.tile([128, 16, COUT], F16, name="ostg")
                # one batched drain per xp
                if xp == 15:
                    nc.vector.tensor_scalar_add(
                        out_sb[g][:, 4 * gs:4 * gs + 4, :],
                        ps[xp][:, 0:4, :], 0.0)
                else:
                    nc.scalar.copy(
                        out_sb[g][:, 4 * gs:4 * gs + 4, :],
                        ps[xp][:, 0:4, :])
                del ps[xp]
                done_in_group[g] += 1
                if g < 3 and done_in_group[g] == 4:
                    nc.scalar.dma_start(o_d[g], out_sb[g][:])
                elif g == 3 and done_in_group[g] == 3:
                    nc.scalar.dma_start(o_d[3][:, 0:12], out_sb[3][:, 0:12])
                elif g == 3 and done_in_group[g] == 4:
                    nc.scalar.dma_start(o_d[3][:, 12:16], out_sb[3][:, 12:16])

    nc.compile()
    return nc


def get_nc():
    global _nc_cache
    if _nc_cache is None:
        _nc_cache = _build_nc()
    return _nc_cache


def prep_inputs(x, weight, bias):
    """Host-side resharding/relayout -> list of 8 per-core input dicts."""
    x = np.asarray(x, dtype=np.float32)
    weight = np.asarray(weight, dtype=np.float32)

    # padded x: [c, Y=34, X=34, b] fp16
    xp_ = np.zeros((CIN, H + 2, H + 2, B), np.float16)
    xp_[:, 1:H + 1, 1:H + 1, :] = x.transpose(1, 2, 3, 0).astype(np.float16)

    ins = []
    for i in range(NCORES):
        s = xp_[:, 4 * i:4 * i + 6, :, :]          # (c, R6, X34, b)
        t = s.transpose(0, 2, 1, 3)                # (c, X34, R6, b)
        xh = np.empty((128, NJ, 6, B), np.float16)
        xh[0:64] = t[:, 0::2, :, :]                # even k
        xh[64:128] = t[:, 1::2, :, :]              # odd k
        xh = np.ascontiguousarray(xh)

        # wm[p, xp, yy, u, v, o]; p = c + 64*half, half by parity rule:
        # even loc -> v%2, odd loc -> (v+1)%2
        wslab = weight[:, :, 4 * i:4 * i + 4, :, :, :]       # (o,c,yy,X,u,v)
        wr = wslab.reshape(COUT, CIN, ROWS, NXP, 2, 3, 3)    # (o,c,yy,xp,par,u,v)
        base = wr.transpose(1, 3, 2, 5, 6, 4, 0)             # (c,xp,yy,u,v,par,o)
        base8 = base.astype(NP_F8)
        wm = np.empty((128, NXP, ROWS, 3, 3, COUT), NP_F8)
        for v in range(3):
            pe = v % 2
            po = (v + 1) % 2
            wm[pe * 64:pe * 64 + 64, :, :, :, v, :] = base8[:, :, :, :, v, 0, :]
            wm[po * 64:po * 64 + 64, :, :, :, v, :] = base8[:, :, :, :, v, 1, :]

        wm3 = np.empty((128, WCOLS), NP_F8)
        for (C, R, v, xp, y0, n, off) in BLOCKS:
            for k in range(n):
                yy = y0 + k
                wm3[:, off + 64 * k: off + 64 * (k + 1)] = wm[:, xp, yy, R - yy, v, :]

        ins.append({"xh": xh, "wm": np.ascontiguousarray(wm3)})
    return ins


def unpack_output(results, bias):
    """results: 8 dicts with 'out_p' [4, 128, 16, 64] -> (B, COUT, H, H)."""
    allout = np.stack([np.asarray(r["out_p"]) for r in results])  # (i,g,p,s,o)
    a = allout.reshape(NCORES, 4, 2, B, 4, ROWS, COUT).astype(np.float32)
    # dims: i g par b gs yy o ; x = 8g + 2gs + par, y = 4i + yy
    t = a.transpose(3, 6, 0, 5, 1, 4, 2)   # (b, o, i, yy, g, gs, par)
    out = t.reshape(B, COUT, H, H)
    out = out + np.asarray(bias, np.float32)[None]
    return np.ascontiguousarray(out)


def kernel(x, weight, bias, _trace=False, _tmpdir=None):
    nc = get_nc()
    in_maps = prep_inputs(x, weight, bias)
    res = run_bass_kernel_spmd(
        nc, in_maps, core_ids=list(range(NCORES)),
        trace=_trace, tmpdir=_tmpdir,
        **({"trace_cores": list(range(NCORES))} if _trace else {}),
    )
    out = unpack_output(res.results, bias)
    if _trace:
        kernel.last_results = res
    return out


# revision 11
# speedup vs baseline: 1.0401x; 1.0401x over previous
"""Locally-connected conv (LocalLinear) Trainium2 Bass kernel.

Problem: x (B=64, Cin=64, 32, 32), weight (Cout=64, Cin=64, 32, 32, 3, 3),
bias (Cout=64, 32, 32) -> out (B=64, Cout=64, 32, 32).
out[b,o,y,x] = sum_{c,u,v} xpad[b,c,y+u-1,x+v-1] * W[o,c,y,x,u,v] + bias[o,y,x]

Sharding: spatial rows across 8 cores (core i owns output rows [4i,4i+4)).

v2 design (vs the 62us fp16 baseline):
  - weights ship as fp8 e3m4 (4 mantissa bits): rel err measured 1.4e-2
    vs the 2e-2 gate; weight DMA halves to 4.72MB/core.  x stays fp16.
  - x ships ONCE (1.67MB): xh[c + 64*(k%2), k//2, R, b] packs even
    padded-columns k in partitions 0-63, odd in 64-127.
  - stationary store X2 [128, R=6, 34*64]: padded column k occupies free
    cols [64k,64k+64) on partition half (k%2); the complement half of
    every column block stays ZERO (pre-zeroed once, split across
    gpsimd/ACT/DVE so no engine serializes the start).  The [128,128]
    stationary for (C,R) is then just the contiguous window
    X2[:, R, 64C:64C+128] - block-diagonal by construction, and each
    x value is stored once (DVE copy volume halved vs the xz scheme).
  - weights are host-gathered into wm3 [128, 36864] fp8 in EXACT matmul
    consumption order: for C ascending, R, v: one moving block
    [128, n*64] covering all valid yy=R-u rows at once (n<=3).  One
    matmul per (C,R,v) - 288 instead of 576 - each streaming a plain
    contiguous wm3 slice.  Moving rows: the weight for tap v of the
    even location sits on partition half (v%2), odd location on the
    complement - matching where X2 holds the corresponding x column.
  - psum: one full bank [128, 8, 64] fp32 per location-pair xp (yy in
    slots 0-3); start=True on the first (C=2xp,R=0) matmul zeroes the
    bank (PSUM zero-region = 2KB), stop on the last (C=2xp+2,R=5).
    One batched ACT drain [128,4,64] per xp (16 drains, not 64).
  - output DMAs ride the gpsimd queue (25ns dispatch, engine idle).
  - input DMAs on the sync queue in consumption order, xh interleaved
    with wm3 column slabs.
"""

import numpy as np
import ml_dtypes

import concourse.bacc as bacc
import concourse.mybir as mybir
import concourse.tile as tile
from concourse.bass_utils import run_bass_kernel_spmd

NCORES = 8
B = 64
CIN = 64
COUT = 64
H = 32
ROWS = 4            # output rows per core
NXP = 16            # location-pairs per row (x paired even/odd)
NCOL = 33           # stationary windows C in [0, 33)
NK = 34             # padded x columns k in [0, 34)
NJ = 17             # xh slots per parity half

F16 = mybir.dt.float16
F8 = mybir.dt.float8e3
F32 = mybir.dt.float32
NP_F8 = ml_dtypes.float8_e3m4

X2W = NK * 64       # X2 free columns (2176)


def _yys(R):
    return range(max(0, R - 2), min(3, R) + 1)


def _blocks():
    """Matmul stream: list of (C, R, v, xp, y0, n, col_off) in order."""
    out = []
    off = 0
    for C in range(NCOL):
        for R in range(6):
            for v in ((0, 2) if C % 2 == 0 else (1,)):
                xp = (C - v) // 2
                if not (0 <= xp < NXP):
                    continue
                ys = list(_yys(R))
                n = len(ys)
                out.append((C, R, v, xp, ys[0], n, off))
                off += n * 64
    return out, off


BLOCKS, WCOLS = _blocks()          # WCOLS == 36864

# input DMA interleave: xh is small (1.67MB) and gates the DVE->PE chain,
# so it is front-loaded; weight slabs stream behind in consumption order.


def _wcol_bound(cbound):
    for (C, R, v, xp, y0, n, off) in BLOCKS:
        if C >= cbound:
            return off
    return WCOLS


_W_SLICES = []
_prev = 0
for cb in (2, 6, 10, 14, 19, 24, 33):
    b = _wcol_bound(cb)
    _W_SLICES.append((_prev, b))
    _prev = b

_nc_cache = None


def _build_nc():
    from contextlib import ExitStack

    nc = bacc.Bacc("TRN2", target_bir_lowering=False)

    xh_d = nc.dram_tensor("xh", [128, NJ, 6, B], F16, kind="ExternalInput")
    w_d = nc.dram_tensor("wm", [128, WCOLS], F8, kind="ExternalInput")
    # out: [4 groups of 4 xp, 128 (b | b+64 odd loc), 16 slots, 64 o] fp16
    o_d = nc.dram_tensor("out_p", [4, 128, 16, COUT], F16, kind="ExternalOutput")

    with tile.TileContext(nc) as tc, ExitStack() as ctx:
        xhpool = ctx.enter_context(tc.tile_pool(name="xhp", bufs=1))
        x2pool = ctx.enter_context(tc.tile_pool(name="x2p", bufs=1))
        wpool = ctx.enter_context(tc.tile_pool(name="wp", bufs=1))
        zpool = ctx.enter_context(tc.tile_pool(name="zp", bufs=1))
        opool = ctx.enter_context(tc.tile_pool(name="op", bufs=4))
        pspool = ctx.enter_context(tc.tile_pool(name="ps", bufs=8, space="PSUM"))

        xh_sb = xhpool.tile([128, NJ, 6, B], F16, name="xh")
        w_sb = wpool.tile([128, WCOLS], F8, name="wm3")
        x2 = x2pool.tile([128, 6, X2W], F16, name="x2")
        zt = zpool.tile([128, 1], F16, name="zt")

        # pre-zero X2: DVE does the front + an early tail chunk (it is idle
        # until the first xh slice lands), ACT broadcast-copies the middle
        # only so its drain queue frees up early.  gpsimd is kept OFF here:
        # concurrent gpsimd memsets were measured to slow overlapping DVE
        # ops ~8x.  The last tail chunk is zeroed mid-stream on DVE.
        nc.vector.memset(zt[:], 0.0)
        nc.vector.memset(x2[:, :, 0:512], 0.0)
        nc.vector.memset(x2[:, :, 1344:1792], 0.0)
        nc.scalar.copy(x2[:, :, 512:1344], zt[:].to_broadcast([128, 6, 832]))

        def copy_slice(j0, j1):
            # xh slots [j0,j1) -> X2 data blocks (even half then odd half)
            nj = j1 - j0
            for h in (0, 1):
                p0 = 64 * h
                src = xh_sb[p0:p0 + 64, j0:j1, :, :]
                dst_ap = x2[p0:p0 + 64, :, :]
                # custom view: [64p, nj (stride 128), 6R (stride X2W), 64b]
                import concourse.ap as cap
                base = x2[p0:p0 + 64, 0, 128 * j0 + 64 * h]
                ap = cap.AP(
                    base.tensor, base.offset,
                    [list(base.ap[0]), [128, nj], [X2W, 6], [1, 64]],
                )
                nc.vector.tensor_scalar_add(ap, src, 0.0)

        # input stream on sync queue: xh front-loaded, weights behind
        def wdma(k):
            a, b = _W_SLICES[k]
            nc.sync.dma_start(w_sb[:, a:b], w_d[:, a:b])

        nc.sync.dma_start(xh_sb[:, 0:1], xh_d[:, 0:1])
        copy_slice(0, 1)
        wdma(0)
        nc.sync.dma_start(xh_sb[:, 1:5], xh_d[:, 1:5])
        copy_slice(1, 5)
        wdma(1)
        nc.sync.dma_start(xh_sb[:, 5:10], xh_d[:, 5:10])
        copy_slice(5, 10)
        wdma(2)
        nc.sync.dma_start(xh_sb[:, 10:17], xh_d[:, 10:17])
        wdma(3)
        nc.vector.memset(x2[:, :, 1792:X2W], 0.0)
        copy_slice(10, 17)
        wdma(4)
        wdma(5)
        wdma(6)

        ps = {}
        out_sb = [None] * 4
        done_in_group = [0] * 4

        for (C, R, v, xp, y0, n, off) in BLOCKS:
            stat = x2[:, R, 64 * C:64 * C + 128]
            mov = w_sb[:, off:off + n * 64]
            first = (v == 0 and R == 0)
            last = (v == 2 and R == 5)
            if first:
                ps[xp] = pspool.tile([128, 8, COUT], F32, name="pst")
            out_ap = ps[xp][:, y0:y0 + n, :]
            nc.tensor.matmul(out_ap, stat, mov, start=first, stop=last)
            if last:
                g, gs = divmod(xp, 4)
                if done_in_group[g] == 0:
                    out_sb[g] = opool.tile([128, 16, COUT], F16, name="ostg")
                # one batched drain per xp
                if xp == 15:
                    nc.vector.tensor_scalar_add(
                        out_sb[g][:, 4 * gs:4 * gs + 4, :],
                        ps[xp][:, 0:4, :], 0.0)
                else:
                    nc.scalar.copy(
                        out_sb[g][:, 4 * gs:4 * gs + 4, :],
                        ps[xp][:, 0:4, :])
                del ps[xp]
                done_in_group[g] += 1
                if g < 3 and done_in_group[g] == 4:
                    nc.scalar.dma_start(o_d[g], out_sb[g][:])
                elif g == 3 and done_in_group[g] == 3:
                    nc.scalar.dma_start(o_d[3][:, 0:12], out_sb[3][:, 0:12])
                elif g == 3 and done_in_group[g] == 4:
                    nc.scalar.dma_start(o_d[3][:, 12:16], out_sb[3][:, 12:16])

    nc.compile()
    return nc


def get_nc():
    global _nc_cache
    if _nc_cache is None:
        _nc_cache = _build_nc()
    return _nc_cache


def prep_inputs(x, weight, bias):
    """Host-side resharding/relayout -> list of 8 per-core input dicts."""
    x = np.asarray(x, dtype=np.float32)
    weight = np.asarray(weight, dtype=np.float32)

    # padded x: [c, Y=34, X=34, b] fp16
    xp_ = np.zeros((CIN, H + 2, H + 2, B), np.float16)
    xp_[:, 1:H + 1, 1:H + 1, :] = x.transpose(1, 2, 3, 0).astype(np.float16)

    ins = []
    for i in range(NCORES):
        s = xp_[:, 4 * i:4 * i + 6, :, :]          # (c, R6, X34, b)
        t = s.transpose(0, 2, 1, 3)                # (c, X34, R6, b)
        xh = np.empty((128, NJ, 6, B), np.float16)
        xh[0:64] = t[:, 0::2, :, :]                # even k
        xh[64:128] = t[:, 1::2, :, :]              # odd k
        xh = np.ascontiguousarray(xh)

        # wm[p, xp, yy, u, v, o]; p = c + 64*half, half by parity rule:
        # even loc -> v%2, odd loc -> (v+1)%2
        wslab = weight[:, :, 4 * i:4 * i + 4, :, :, :]       # (o,c,yy,X,u,v)
        wr = wslab.reshape(COUT, CIN, ROWS, NXP, 2, 3, 3)    # (o,c,yy,xp,par,u,v)
        base = wr.transpose(1, 3, 2, 5, 6, 4, 0)             # (c,xp,yy,u,v,par,o)
        base8 = base.astype(NP_F8)
        wm = np.empty((128, NXP, ROWS, 3, 3, COUT), NP_F8)
        for v in range(3):
            pe = v % 2
            po = (v + 1) % 2
            wm[pe * 64:pe * 64 + 64, :, :, :, v, :] = base8[:, :, :, :, v, 0, :]
            wm[po * 64:po * 64 + 64, :, :, :, v, :] = base8[:, :, :, :, v, 1, :]

        wm3 = np.empty((128, WCOLS), NP_F8)
        for (C, R, v, xp, y0, n, off) in BLOCKS:
            for k in range(n):
                yy = y0 + k
                wm3[:, off + 64 * k: off + 64 * (k + 1)] = wm[:, xp, yy, R - yy, v, :]

        ins.append({"xh": xh, "wm": np.ascontiguousarray(wm3)})
    return ins


def unpack_output(results, bias):
    """results: 8 dicts with 'out_p' [4, 128, 16, 64] -> (B, COUT, H, H)."""
    allout = np.stack([np.asarray(r["out_p"]) for r in results])  # (i,g,p,s,o)
    a = allout.reshape(NCORES, 4, 2, B, 4, ROWS, COUT).astype(np.float32)
    # dims: i g par b gs yy o ; x = 8g + 2gs + par, y = 4i + yy
    t = a.transpose(3, 6, 0, 5, 1, 4, 2)   # (b, o, i, yy, g, gs, par)
    out = t.reshape(B, COUT, H, H)
    out = out + np.asarray(bias, np.float32)[None]
    return np.ascontiguousarray(out)


def kernel(x, weight, bias, _trace=False, _tmpdir=None):
    nc = get_nc()
    in_maps = prep_inputs(x, weight, bias)
    res = run_bass_kernel_spmd(
        nc, in_maps, core_ids=list(range(NCORES)),
        trace=_trace, tmpdir=_tmpdir,
        **({"trace_cores": list(range(NCORES))} if _trace else {}),
    )
    out = unpack_output(res.results, bias)
    if _trace:
        kernel.last_results = res
    return out
